# revision 1
# baseline (speedup 1.0000x reference)
"""Trainium2 Bass kernel for nn_InputRotationWrapper: y = WHT(x) @ W^T + b.

Algebraic fold: WHT (normalized Walsh-Hadamard along feature dim, H symmetric)
commutes into the weight: y = (x H) W^T = x (W H)^T.  The device therefore runs
a pure GEMM  y = x @ Wr^T + b  with Wr = WHT(W) computed once on the host.

Distribution: data-parallel over the 8192 tokens across 8 NeuronCores (1024
tokens each); Wr is replicated.  Each core computes its output slice
transposed (yT[o, t], o on partitions) so every DMA is fully contiguous:
  - x^T shard  [128 p, 32 c, 1024 t]  resident in SBUF as fp16 (8.4 MB)
  - Wr packed  [32 ob, 128 d_in, 32 d_chunk, 128 o]  fp16, streamed per o-block
  - out yT     [4096 o, 1024 t]  fp16, written per o-block (host re-widens)

Matmul dtype is float16: full PE rate (1 row/cycle), and the per-matmul
LDWEIGHTS (128x128 stationary tile) takes ~100ns at 2B/row vs fp32r's 224ns,
so it hides completely in the PE shadow weight buffer behind the previous
512-row matmul (213ns).  fp32r's 224+44ns load path gated the original
kernel at a 272ns cadence; fp16 runs at the ~216ns roofline cadence.

Startup is HBM-arrival-bound (the DMA subsystem ramps from ~85 GB/s to
~400 GB/s over the first few us): x streams on the Scalar HWDGE queue in
size-ramped pieces while the warmup W quarters stream on Sync, both in
arrival-need order.  The warmup group runs c-outer over 4 o-blocks (all 8
PSUM banks) so each arriving x chunk immediately unlocks 8 matmuls, and
finishes block-major so its evictions stagger.  Bias is fused into the
PSUM->SBUF eviction via ScalarE activation.  The last o-block runs
token-half 0 fully before half 1 (separate PSUM tiles — a shared tile
would serialize on a whole-tile WAR edge) so only one half-eviction tails.
"""
import sys

for _p in ("/opt/trn_rl_repo", "/root/.axon_site/_ro/trn_rl_repo"):
    if _p not in sys.path:
        sys.path.insert(0, _p)

import numpy as np

D = 4096          # feature dim (= rotation size)
TOKENS = 8192     # 4 * 2048
N_CORES = 8
T_CORE = TOKENS // N_CORES   # 1024 tokens per core
P = 128           # partitions
DC = D // P       # 32 contraction chunks
OB = D // P       # 32 output blocks
T_HALF = 512      # moving free-dim per matmul (hw max)

_compiled = None


def _matmul_hadU_np(x: np.ndarray) -> np.ndarray:
    """Normalized WHT along the last axis — exact port of the reference
    recursive-butterfly (K == 1 branch), in float64."""
    n = x.shape[-1]
    shape = x.shape
    v = x.reshape(-1, n, 1)
    while v.shape[1] > 1:
        b_, m, c = v.shape
        v = v.reshape(b_, m // 2, 2, c)
        a, b = v[:, :, 0, :], v[:, :, 1, :]
        v = np.concatenate([a + b, a - b], axis=-1)
    return v.reshape(shape) / np.sqrt(n)


def _build_nc():
    import concourse.tile as tile
    from concourse import bacc, mybir

    dt = mybir.dt
    nc = bacc.Bacc(None, target_bir_lowering=False)

    xt_d = nc.dram_tensor("xt", [P, DC, T_CORE], dt.float16, kind="ExternalInput")
    w_d = nc.dram_tensor("w", [OB, P, DC, P], dt.float16, kind="ExternalInput")
    b_d = nc.dram_tensor("bias", [P, OB], dt.float32, kind="ExternalInput")
    y_d = nc.dram_tensor("yt", [D, T_CORE], dt.float16, kind="ExternalOutput")

    G0 = 4   # o-blocks processed c-outer in the startup group: 8 matmuls
             # become ready per arriving x chunk, saturating the PE while the
             # 8.4 MB x shard streams in.  Uses all 8 PSUM banks.
    QC = 8   # startup W granularity: quarter-tiles of 8 contraction chunks
    HC = 16  # steady-state W granularity: half-tiles (fewer DMA triggers)
    NQ = DC // QC
    PRE = 3  # steady blocks whose W is prefetched on Sync during startup

    with tile.TileContext(nc) as tc:
        with (
            tc.tile_pool(name="xp", bufs=1) as xp,
            tc.tile_pool(name="wqp", bufs=G0 * NQ, space="SBUF") as wqp,
            tc.tile_pool(name="whp", bufs=2 * (PRE + 1), space="SBUF") as whp,
            tc.tile_pool(name="bp", bufs=1) as bp,
            tc.tile_pool(name="op", bufs=4) as op,
            tc.tile_pool(name="pp", bufs=G0, space="PSUM") as pp,
        ):
            b_sb = bp.tile([P, OB], dt.float32)

            ps0 = [
                pp.tile([P, T_CORE], dt.float32, tag="ps", name=f"ps0_{i}")
                for i in range(G0)
            ]

            # PE warm-up: the PE clock p-state drops when idle and takes
            # ~3us of activity to ramp back, and the engines sit idle from
            # the end of the preamble (~7.3us) until the first x/W bytes
            # land (~12.2us).  Burn ~3us of that dead window with dummy
            # matmuls (~213ns each) into a PSUM region the real c=0
            # accumulation resets (start=True), pulling the ramp off the
            # critical path without delaying real work if DMA runs slow.
            dum = bp.tile([P, 256], dt.float16, tag="dum", name="dum")
            nc.vector.memset(dum[:], 0.0)
            for _ in range(14):
                nc.tensor.matmul(
                    ps0[0][:, 0:256], dum[:, 0:128], dum[:, 0:256],
                    start=True, stop=True,
                )

            def load_wq(ob, q, eng=None):
                wq = wqp.tile([P, QC, P], dt.float16, tag="w",
                              name=f"w_{ob}_{q}")
                (eng or nc.sync).dma_start(
                    wq[:], w_d[ob, :, q * QC:(q + 1) * QC, :])
                return wq

            def load_wh(ob, h):
                wh = whp.tile([P, HC, P], dt.float16, tag="w2", name=f"wh_{ob}_{h}")
                nc.sync.dma_start(wh[:], w_d[ob, :, h * HC:(h + 1) * HC, :])
                return wh

            # per-chunk (half0_ap, half1_ap) matmul operand pairs
            x_half = [None] * DC

            def load_x_group(c0, n):
                t = xp.tile([P, n, T_CORE], dt.float16, tag=f"x{c0}",
                            name=f"x_{c0}")
                nc.scalar.dma_start(t[:], xt_d[:, c0:c0 + n, :])
                for i in range(n):
                    x_half[c0 + i] = (t[:, i, 0:T_HALF],
                                      t[:, i, T_HALF:T_CORE])

            def mms(ps, lhsT, c):
                xh = x_half[c]
                nc.tensor.matmul(
                    ps[:, 0:T_HALF], lhsT, xh[0],
                    start=(c == 0), stop=(c == DC - 1),
                )
                nc.tensor.matmul(
                    ps[:, T_HALF:T_CORE], lhsT, xh[1],
                    start=(c == 0), stop=(c == DC - 1),
                )


            def evict_half(ob, ps, h):
                sl = slice(h * T_HALF, (h + 1) * T_HALF)
                o_sb = op.tile([P, T_HALF], dt.float16, tag="o",
                               name=f"o_{ob}_{h}")
                nc.scalar.activation(
                    o_sb[:], ps[:, sl],
                    mybir.ActivationFunctionType.Identity,
                    bias=b_sb[:, ob:ob + 1],
                )
                nc.sync.dma_start(y_d[ob * P:(ob + 1) * P, sl], o_sb[:])

            # Scalar HWDGE queue carries x in need order with sizes ramping
            # up; Sync carries the W quarters.  Trigger order approximates
            # arrival-need order across both queues.
            w0 = [[None] * NQ for _ in range(G0)]
            load_x_group(0, 1)
            for ob in range(G0):
                w0[ob][0] = load_wq(ob, 0)
            load_x_group(1, 1)
            load_x_group(2, 1)
            load_x_group(3, 1)
            for ob in range(G0):
                w0[ob][1] = load_wq(ob, 1)
            load_x_group(4, 2)
            load_x_group(6, 2)
            load_x_group(8, 4)
            for ob in range(G0):
                w0[ob][2] = load_wq(ob, 2)
            load_x_group(12, 4)
            load_x_group(16, 4)
            load_x_group(20, 4)
            load_x_group(24, 4)
            load_x_group(28, 4)
            # q3 rides at the tail of the Scalar x stream instead of Sync:
            # 25% less W contention during the bandwidth-starved early
            # window, and in-order completion still lands it (~35us) far
            # ahead of its first consumer at c=24 (~53us).
            for ob in range(G0):
                w0[ob][3] = load_wq(ob, 3, eng=nc.scalar)

            # Sync queue: bias, then W for the first steady blocks so block
            # G0 starts without waiting on the eviction-gated trigger chain.
            nc.sync.dma_start(b_sb[:], b_d[:])
            whs = {}
            for ob in range(G0, G0 + PRE):
                whs[ob] = [load_wh(ob, h) for h in range(2)]

            # c-outer while x streams in; the last 4 c-steps go block-major
            # so startup blocks finish staggered and their evictions (which
            # free the PSUM ring for the first steady blocks) pipeline with
            # the remaining matmuls instead of bunching at the end.
            C_SPLIT = DC - 4
            for c in range(C_SPLIT):
                for ob in range(G0):
                    mms(ps0[ob], w0[ob][c // QC][:, c % QC, :], c)
            for ob in range(G0):
                for c in range(C_SPLIT, DC):
                    mms(ps0[ob], w0[ob][c // QC][:, c % QC, :], c)
                evict_half(ob, ps0[ob], 0)
                evict_half(ob, ps0[ob], 1)

            # steady state: one o-block at a time, W halves prefetched
            for ob in range(G0, OB):
                if ob not in whs:
                    whs[ob] = [load_wh(ob, h) for h in range(2)]
                nxt = ob + PRE
                if G0 + PRE <= nxt < OB:
                    whs[nxt] = [load_wh(nxt, h) for h in range(2)]
                if ob < OB - 1:
                    ps = pp.tile([P, T_CORE], dt.float32, tag="ps",
                                 name=f"ps_{ob}")
                    for c in range(DC):
                        mms(ps, whs[ob][c // HC][:, c % HC, :], c)
                    evict_half(ob, ps, 0)
                    evict_half(ob, ps, 1)
                else:
                    # last block: finish token-half 0 first so its eviction
                    # overlaps half 1's matmuls; only half 1's eviction
                    # tails.  Separate PSUM tiles per half — with a shared
                    # tile the h1 matmuls pick up a whole-tile WAR edge on
                    # the h0 eviction read and stall ~1.2us.
                    for h in range(2):
                        psh = pp.tile([P, T_CORE], dt.float32, tag="ps",
                                      name=f"ps_{ob}_{h}")
                        sl = slice(h * T_HALF, (h + 1) * T_HALF)
                        for c in range(DC):
                            nc.tensor.matmul(
                                psh[:, sl], whs[ob][c // HC][:, c % HC, :],
                                x_half[c][h],
                                start=(c == 0), stop=(c == DC - 1),
                            )
                        evict_half(ob, psh, h)

    nc.compile()
    return nc


def _get_nc():
    global _compiled
    if _compiled is None:
        _compiled = _build_nc()
    return _compiled


def _prep_inputs(x, W, b):
    x = np.asarray(x, dtype=np.float32)
    W = np.asarray(W, dtype=np.float32)
    b = np.asarray(b, dtype=np.float32)

    Wr = _matmul_hadU_np(W.astype(np.float64))  # [o, d] float64
    # W_pack[ob, p, c, j] = Wr[ob*128 + j, c*128 + p]
    w_pack = np.ascontiguousarray(
        Wr.reshape(OB, P, DC, P).transpose(0, 3, 2, 1).astype(np.float16)
    )
    b_pack = np.ascontiguousarray(b.reshape(OB, P).T)  # [128, 32]

    # xt[core, p, c, t] = x_core^T[c*128 + p, t]: partition-major so each
    # multi-chunk DMA reads one contiguous span per partition.
    xt = np.ascontiguousarray(
        x.reshape(N_CORES, T_CORE, D).transpose(0, 2, 1).astype(np.float16)
        .reshape(N_CORES, DC, P, T_CORE).transpose(0, 2, 1, 3)
    )

    in_maps = [
        {"xt": xt[c], "w": w_pack, "bias": b_pack} for c in range(N_CORES)
    ]
    return in_maps


def _assemble(results):
    # yt per core: [4096 o, 1024 t] fp16 -> y[t, o] fp32
    parts = [r["yt"].T.astype(np.float32) for r in results]
    y = np.concatenate(parts, axis=0)  # [8192, 4096]
    return y.reshape(4, 2048, D)


def _run(x, W, b, **spmd_kwargs):
    from concourse.bass_utils import run_bass_kernel_spmd

    nc = _get_nc()
    in_maps = _prep_inputs(x, W, b)
    res = run_bass_kernel_spmd(nc, in_maps, list(range(N_CORES)), **spmd_kwargs)
    return _assemble(res.results), res


def kernel(x, W, b):
    out, _ = _run(x, W, b)
    return out



# revision 5
# speedup vs baseline: 1.0459x; 1.0459x over previous
"""Trainium2 Bass kernel for nn_InputRotationWrapper: y = WHT(x) @ W^T + b.

Algebraic fold: WHT (normalized Walsh-Hadamard along feature dim, H symmetric)
commutes into the weight: y = (x H) W^T = x (W H)^T.  The device runs a pure
GEMM  y = x @ Wr^T + b  with Wr = WHT(W) computed once on the host.

On top of the fold, one level of STRASSEN over 2x2x2 blocking of
(o, k, t) cuts the PE matmul count by 1/8 — the kernel is PE-streaming-bound
at fp16 (1 moving column/cycle), so this is a direct 12.5% win that neither
fp8 (accuracy: e4m3 x,W measures 3.8e-2 rel err vs the 2e-2 gate) nor uint8
(TRN2 silicon zeroes integer matmul products; probed via NEFF dtype patch)
can reach.

  C = Wr @ x^T = [[C11 C12],[C21 C22]],  A = Wr halves, B = x^T halves
  M1=(A11+A22)(B11+B22) M2=(A21+A22)B11 M3=A11(B12-B22) M4=A22(B21-B11)
  M5=(A11+A12)B22 M6=(A21-A11)(B11+B12) M7=(A12-A22)(B21+B22)
  C11=M1+M4-M5+M7  C12=M3+M5  C21=M2+M4  C22=M1-M2+M3+M6

All 7 A-combos (W-side) and 7 B-combos (x-side) are precomputed on the host
in f64/f32 and shipped as fp16: the device only runs products and cheap
recombines.  Per core (1024 tokens, data-parallel over 8 cores):

  - 7 x-combos resident in SBUF: [128p, 16c, 512t] fp16 each (14.7 MB)
  - W-combos streamed per (product j, o-block obp): [128p, 16c, 128o] fp16
  - 16 obp iterations x 7 products x 16-chunk PSUM accumulation
    = 1792 matmuls of 512 cols (vs 2048 classical) ~ 387 us PE wall
  - ScalarE evicts each product PSUM->SBUF fp16; VectorE recombines with
    scalar_tensor_tensor (bias fused via the per-partition scalar operand);
    outputs DMA per [128, 512] slice.  All hidden under PE time.

Startup mirrors the fp16 baseline: PE-clock warmup dummies, then a j-major
group over the first G o-blocks processed c-outer so every arriving x-combo
chunk immediately unlocks G matmuls while the DMA subsystem ramps.
"""
import sys

for _p in ("/opt/trn_rl_repo", "/root/.axon_site/_ro/trn_rl_repo"):
    if _p not in sys.path:
        sys.path.insert(0, _p)

import numpy as np

D = 4096          # feature dim (= rotation size)
TOKENS = 8192     # 4 * 2048
N_CORES = 8
T_CORE = TOKENS // N_CORES   # 1024 tokens per core
P = 128           # partitions
HALF = D // 2     # 2048: o/k half size
KH = HALF // P    # 16 contraction chunks per half
OBH = HALF // P   # 16 output blocks per half
TH = T_CORE // 2  # 512 tokens per t-half (= one matmul moving dim)
NPROD = 7

_compiled = None


def _matmul_hadU_np(x: np.ndarray) -> np.ndarray:
    """Normalized WHT along the last axis — exact port of the reference
    recursive-butterfly (K == 1 branch), in float64."""
    n = x.shape[-1]
    shape = x.shape
    v = x.reshape(-1, n, 1)
    while v.shape[1] > 1:
        b_, m, c = v.shape
        v = v.reshape(b_, m // 2, 2, c)
        a, b = v[:, :, 0, :], v[:, :, 1, :]
        v = np.concatenate([a + b, a - b], axis=-1)
    return v.reshape(shape) / np.sqrt(n)


def _build_nc():
    import concourse.tile as tile
    from concourse import bacc, mybir

    dt = mybir.dt
    alu = mybir.AluOpType
    nc = bacc.Bacc(None, target_bir_lowering=False)

    xc_d = nc.dram_tensor("xc", [NPROD, P, KH, TH], dt.float16,
                          kind="ExternalInput")
    wc_d = nc.dram_tensor("wc", [NPROD, OBH, P, KH, P], dt.float16,
                          kind="ExternalInput")
    b_d = nc.dram_tensor("bias", [P, 2 * OBH], dt.float32,
                         kind="ExternalInput")
    y_d = nc.dram_tensor("yt", [D, T_CORE], dt.float16, kind="ExternalOutput")

    G = 3     # startup group: o-blocks processed c-outer per product so each
              # arriving x-combo chunk unlocks G matmuls during the DMA ramp
    WRING = 8   # W tile ring (4 KB/partition each)
    MRING = 7 * G + 7  # staged-product ring (1 KB/partition each)

    with tile.TileContext(nc) as tc:
        with (
            tc.tile_pool(name="xcp", bufs=1) as xcp,
            tc.tile_pool(name="wp", bufs=WRING) as wp,
            tc.tile_pool(name="mp", bufs=MRING) as mp,
            tc.tile_pool(name="tp", bufs=4) as tp,
            tc.tile_pool(name="op", bufs=6) as op,
            tc.tile_pool(name="bp", bufs=1) as bp,
            tc.tile_pool(name="pp", bufs=8, space="PSUM") as pp,
        ):
            b_sb = bp.tile([P, 2 * OBH], dt.float32)

            xc_sb = [
                xcp.tile([P, KH, TH], dt.float16, name=f"xc_{j}")
                for j in range(NPROD)
            ]

            # ---- PE clock warmup (HAM ramps over ~3.4us of activity) ----
            dum = bp.tile([P, 256], dt.float16, tag="dum", name="dum")
            nc.vector.memset(dum[:], 0.0)

            w_tiles = {}

            def w_alloc(j, obp):
                t = wp.tile([P, KH, P], dt.float16, tag="w",
                            name=f"w_{j}_{obp}")
                w_tiles[(j, obp)] = t
                return t

            def w_load(j, obp, eng=None):
                t = w_alloc(j, obp)
                (eng or nc.gpsimd).dma_start(t[:], wc_d[j, obp, :, :, :])
                return t

            def xc_load(j, c0, n, eng=None):
                (eng or nc.scalar).dma_start(
                    xc_sb[j][:, c0:c0 + n, :], xc_d[j, :, c0:c0 + n, :])

            # ---- DMA triggers in arrival-need order ----
            # scalar queue: x-combos, ramped sizes; gpsimd queue: W tiles;
            # sync queue: bias + output slices.
            xc_load(0, 0, 1)
            for gob in range(G):
                t = w_alloc(0, gob)
                nc.gpsimd.dma_start(t[:, 0:4, :], wc_d[0, gob, :, 0:4, :])
            xc_load(0, 1, 1)
            xc_load(0, 2, 1)
            xc_load(0, 3, 1)
            for gob in range(G):
                nc.gpsimd.dma_start(
                    w_tiles[(0, gob)][:, 4:8, :], wc_d[0, gob, :, 4:8, :])
            xc_load(0, 4, 2)
            xc_load(0, 6, 2)
            for gob in range(G):
                nc.gpsimd.dma_start(
                    w_tiles[(0, gob)][:, 8:16, :], wc_d[0, gob, :, 8:16, :])
            xc_load(0, 8, 4)
            xc_load(0, 12, 4)
            nc.sync.dma_start(b_sb[:], b_d[:])
            xc_load(1, 0, 4)
            xc_load(1, 4, 4)
            for j in range(1, NPROD):
                for gob in range(G):
                    w_load(j, gob)
            xc_load(1, 8, 8)
            xc_load(2, 0, 8)
            xc_load(2, 8, 8)
            for j in range(3, NPROD):
                xc_load(j, 0, 16)

            # startup W for the first steady blocks so obp=G starts clean
            for j in range(NPROD):
                w_load(j, G)

            # ---- PE warmup dummies ----
            ps_warm = pp.tile([P, TH], dt.float32, tag="ps", name="ps_w")
            for _ in range(14):
                nc.tensor.matmul(
                    ps_warm[:, 0:256], dum[:, 0:128], dum[:, 0:256],
                    start=True, stop=True,
                )

            stage = {}

            def evict(j, obp, ps):
                m = mp.tile([P, TH], dt.float16, tag="m", name=f"m_{j}_{obp}")
                nc.scalar.copy(m[:], ps[:])
                stage[(j, obp)] = m
                return m

            def product(j, obp, ps=None):
                if ps is None:
                    ps = pp.tile([P, TH], dt.float32, tag="ps",
                                 name=f"ps_{j}_{obp}")
                wt = w_tiles.pop((j, obp))
                for c in range(KH):
                    nc.tensor.matmul(
                        ps[:], wt[:, c, :], xc_sb[j][:, c, :],
                        start=(c == 0), stop=(c == KH - 1),
                    )
                evict(j, obp, ps)

            def recombine(obp):
                bt = b_sb[:, obp:obp + 1]
                bb = b_sb[:, OBH + obp:OBH + obp + 1]
                m = [stage.pop((j, obp)) for j in range(NPROD)]
                v = nc.vector
                t1 = tp.tile([P, TH], dt.float16, tag="t", name=f"t1_{obp}")
                t2 = tp.tile([P, TH], dt.float16, tag="t", name=f"t2_{obp}")
                t3 = tp.tile([P, TH], dt.float16, tag="t", name=f"t3_{obp}")
                t4 = tp.tile([P, TH], dt.float16, tag="t", name=f"t4_{obp}")
                o11 = op.tile([P, TH], dt.float16, tag="o", name=f"o11_{obp}")
                o12 = op.tile([P, TH], dt.float16, tag="o", name=f"o12_{obp}")
                o21 = op.tile([P, TH], dt.float16, tag="o", name=f"o21_{obp}")
                o22 = op.tile([P, TH], dt.float16, tag="o", name=f"o22_{obp}")
                # C11 = M1+M4-M5+M7+bt   C12 = M3+M5+bt
                # C21 = M2+M4+bb         C22 = M1-M2+M3+M6+bb
                v.scalar_tensor_tensor(t1[:], m[0][:], bt, m[3][:],
                                       alu.add, alu.add)
                v.scalar_tensor_tensor(t2[:], m[6][:], 0.0, m[4][:],
                                       alu.add, alu.subtract)
                v.scalar_tensor_tensor(o11[:], t1[:], 0.0, t2[:],
                                       alu.add, alu.add)
                v.scalar_tensor_tensor(o12[:], m[2][:], bt, m[4][:],
                                       alu.add, alu.add)
                v.scalar_tensor_tensor(t3[:], m[0][:], bb, m[1][:],
                                       alu.add, alu.subtract)
                v.scalar_tensor_tensor(t4[:], m[2][:], 0.0, m[5][:],
                                       alu.add, alu.add)
                v.scalar_tensor_tensor(o22[:], t3[:], 0.0, t4[:],
                                       alu.add, alu.add)
                v.scalar_tensor_tensor(o21[:], m[1][:], bb, m[3][:],
                                       alu.add, alu.add)
                rt = slice(obp * P, (obp + 1) * P)
                rb = slice((OBH + obp) * P, (OBH + obp + 1) * P)
                nc.sync.dma_start(y_d[rt, 0:TH], o11[:])
                nc.sync.dma_start(y_d[rt, TH:T_CORE], o12[:])
                nc.sync.dma_start(y_d[rb, 0:TH], o21[:])
                nc.sync.dma_start(y_d[rb, TH:T_CORE], o22[:])

            # ---- startup group: j-major, c-outer across obp 0..G-1 ----
            for j in range(NPROD):
                ps_j = []
                for gob in range(G):
                    if j == 0 and gob == 0:
                        ps_j.append(ps_warm)
                    else:
                        ps_j.append(pp.tile(
                            [P, TH], dt.float32, tag="ps",
                            name=f"ps_{j}_{gob}"))
                for c in range(KH):
                    for gob in range(G):
                        nc.tensor.matmul(
                            ps_j[gob][:],
                            w_tiles[(j, gob)][:, c, :], xc_sb[j][:, c, :],
                            start=(c == 0), stop=(c == KH - 1),
                        )
                for gob in range(G):
                    evict(j, gob, ps_j[gob])
            for j, gob in list(w_tiles):
                if gob < G:
                    del w_tiles[(j, gob)]
            for gob in range(G):
                recombine(gob)

            # ---- steady state: obp-major ----
            for obp in range(G, OBH):
                for j in range(NPROD):
                    if obp + 1 < OBH:
                        w_load(j, obp + 1)
                    product(j, obp)
                recombine(obp)

    nc.compile()
    return nc


def _get_nc():
    global _compiled
    if _compiled is None:
        _compiled = _build_nc()
    return _compiled


def _prep_inputs(x, W, b):
    x = np.asarray(x, dtype=np.float32)
    W = np.asarray(W, dtype=np.float32)
    b = np.asarray(b, dtype=np.float32)

    Wr = _matmul_hadU_np(W.astype(np.float64))  # [o, k] float64
    A11 = Wr[:HALF, :HALF]
    A12 = Wr[:HALF, HALF:]
    A21 = Wr[HALF:, :HALF]
    A22 = Wr[HALF:, HALF:]
    WCs = (A11 + A22, A21 + A22, A11, A22, A11 + A12, A21 - A11, A12 - A22)
    # pack[j][obp, p, c, jo] = WC_j[obp*128 + jo, c*128 + p]
    wc = np.stack([
        w.reshape(OBH, P, KH, P).transpose(0, 3, 2, 1) for w in WCs
    ]).astype(np.float16)
    wc = np.ascontiguousarray(wc)

    b_pack = np.ascontiguousarray(b.reshape(2 * OBH, P).T)  # [128, 32]

    xt = x.reshape(N_CORES, T_CORE, D).transpose(0, 2, 1)  # [core, k, t] f32
    B11 = xt[:, :HALF, :TH]
    B12 = xt[:, :HALF, TH:]
    B21 = xt[:, HALF:, :TH]
    B22 = xt[:, HALF:, TH:]
    XCs = (B11 + B22, B11, B12 - B22, B21 - B11, B22, B11 + B12, B21 + B22)
    # pack[core, j, p, c, t] = XC_j[core, c*128 + p, t]
    xc = np.stack([
        c.reshape(N_CORES, KH, P, TH).transpose(0, 2, 1, 3) for c in XCs
    ], axis=1).astype(np.float16)
    xc = np.ascontiguousarray(xc)

    in_maps = [
        {"xc": xc[i], "wc": wc, "bias": b_pack} for i in range(N_CORES)
    ]
    return in_maps


def _assemble(results):
    # yt per core: [4096 o, 1024 t] fp16 -> y[t, o] fp32
    parts = [r["yt"].T.astype(np.float32) for r in results]
    y = np.concatenate(parts, axis=0)  # [8192, 4096]
    return y.reshape(4, 2048, D)


def _run(x, W, b, **spmd_kwargs):
    from concourse.bass_utils import run_bass_kernel_spmd

    nc = _get_nc()
    in_maps = _prep_inputs(x, W, b)
    res = run_bass_kernel_spmd(nc, in_maps, list(range(N_CORES)), **spmd_kwargs)
    return _assemble(res.results), res


def kernel(x, W, b):
    out, _ = _run(x, W, b)
    return out


# revision 13
# speedup vs baseline: 1.0521x; 1.0059x over previous
"""Trainium2 Bass kernel for nn_InputRotationWrapper: y = WHT(x) @ W^T + b.

Algebraic fold: WHT (normalized Walsh-Hadamard along feature dim, H symmetric)
commutes into the weight: y = (x H) W^T = x (W H)^T.  The device runs a pure
GEMM  y = x @ Wr^T + b  with Wr = WHT(W) computed once on the host.

On top of the fold, one level of STRASSEN over 2x2x2 blocking of
(o, k, t) cuts the PE matmul count by 1/8 — the kernel is PE-streaming-bound
at fp16 (1 moving column/cycle), so this is a direct 12.5% win that neither
fp8 (accuracy: e4m3 x,W measures 3.8e-2 rel err vs the 2e-2 gate) nor uint8
(TRN2 silicon zeroes integer matmul products; probed via NEFF dtype patch)
can reach.

  C = Wr @ x^T = [[C11 C12],[C21 C22]],  A = Wr halves, B = x^T halves
  M1=(A11+A22)(B11+B22) M2=(A21+A22)B11 M3=A11(B12-B22) M4=A22(B21-B11)
  M5=(A11+A12)B22 M6=(A21-A11)(B11+B12) M7=(A12-A22)(B21+B22)
  C11=M1+M4-M5+M7  C12=M3+M5  C21=M2+M4  C22=M1-M2+M3+M6

All 7 A-combos (W-side) and 7 B-combos (x-side) are precomputed on the host
in f64/f32 and shipped as fp16: the device only runs products and cheap
recombines.  Per core (1024 tokens, data-parallel over 8 cores):

  - 7 x-combos resident in SBUF: [128p, 16c, 512t] fp16 each (14.7 MB)
  - W-combos streamed per (product j, o-block obp): [128p, 16c, 128o] fp16
  - 16 obp iterations x 7 products x 16-chunk PSUM accumulation
    = 1792 matmuls of 512 cols (vs 2048 classical) ~ 387 us PE wall
  - ScalarE evicts each product PSUM->SBUF fp16; VectorE recombines with
    scalar_tensor_tensor (bias fused via the per-partition scalar operand);
    outputs DMA per [128, 512] slice.  All hidden under PE time.

Startup mirrors the fp16 baseline: PE-clock warmup dummies, then a j-major
group over the first G o-blocks processed c-outer so every arriving x-combo
chunk immediately unlocks G matmuls while the DMA subsystem ramps.
"""
import sys

for _p in ("/opt/trn_rl_repo", "/root/.axon_site/_ro/trn_rl_repo"):
    if _p not in sys.path:
        sys.path.insert(0, _p)

import numpy as np

D = 4096          # feature dim (= rotation size)
TOKENS = 8192     # 4 * 2048
N_CORES = 8
T_CORE = TOKENS // N_CORES   # 1024 tokens per core
P = 128           # partitions
HALF = D // 2     # 2048: o/k half size
KH = HALF // P    # 16 contraction chunks per half
OBH = HALF // P   # 16 output blocks per half
TH = T_CORE // 2  # 512 tokens per t-half (= one matmul moving dim)
NPROD = 7

_compiled = None


def _matmul_hadU_np(x: np.ndarray) -> np.ndarray:
    """Normalized WHT along the last axis — exact port of the reference
    recursive-butterfly (K == 1 branch), in float64."""
    n = x.shape[-1]
    shape = x.shape
    v = x.reshape(-1, n, 1)
    while v.shape[1] > 1:
        b_, m, c = v.shape
        v = v.reshape(b_, m // 2, 2, c)
        a, b = v[:, :, 0, :], v[:, :, 1, :]
        v = np.concatenate([a + b, a - b], axis=-1)
    return v.reshape(shape) / np.sqrt(n)


def _build_nc():
    import concourse.tile as tile
    from concourse import bacc, mybir

    dt = mybir.dt
    alu = mybir.AluOpType
    nc = bacc.Bacc(None, target_bir_lowering=False)

    xc_d = nc.dram_tensor("xc", [NPROD, P, KH, TH], dt.float16,
                          kind="ExternalInput")
    wc_d = nc.dram_tensor("wc", [NPROD, OBH, P, KH, P], dt.float16,
                          kind="ExternalInput")
    b_d = nc.dram_tensor("bias", [P, 2 * OBH], dt.float32,
                         kind="ExternalInput")
    y_d = nc.dram_tensor("yt", [D, T_CORE], dt.float16, kind="ExternalOutput")

    G = 4     # startup group: o-blocks processed c-outer per product so each
              # arriving x-combo chunk unlocks G matmuls during the DMA ramp
    WRING = 8   # W tile ring (4 KB/partition each)
    MRING = 24  # staged-product ring (1 KB/partition each); incremental
                # recombine frees most of the startup group by j=3

    with tile.TileContext(nc) as tc:
        with (
            tc.tile_pool(name="xcp", bufs=1) as xcp,
            tc.tile_pool(name="wp", bufs=WRING) as wp,
            tc.tile_pool(name="mp", bufs=MRING) as mp,
            tc.tile_pool(name="tp", bufs=12) as tp,
            tc.tile_pool(name="op", bufs=4) as op,
            tc.tile_pool(name="bp", bufs=1) as bp,
            tc.tile_pool(name="pp", bufs=8, space="PSUM") as pp,
        ):
            b_sb = bp.tile([P, 2 * OBH], dt.float32)

            xc_sb = [
                xcp.tile([P, KH, TH], dt.float16, name=f"xc_{j}")
                for j in range(NPROD)
            ]

            # ---- PE clock warmup (HAM ramps over ~3.4us of activity) ----
            dum = bp.tile([P, 256], dt.float16, tag="dum", name="dum")
            nc.vector.memset(dum[:], 0.0)

            w_tiles = {}

            def w_alloc(j, obp):
                t = wp.tile([P, KH, P], dt.float16, tag="w",
                            name=f"w_{j}_{obp}")
                w_tiles[(j, obp)] = t
                return t

            def w_load(j, obp, eng=None):
                t = w_alloc(j, obp)
                (eng or nc.gpsimd).dma_start(t[:], wc_d[j, obp, :, :, :])
                return t

            def xc_load(j, c0, n, eng=None):
                (eng or nc.scalar).dma_start(
                    xc_sb[j][:, c0:c0 + n, :], xc_d[j, :, c0:c0 + n, :])

            # ---- DMA triggers in arrival-need order ----
            # scalar queue: x-combos, ramped sizes; gpsimd queue: W tiles;
            # sync queue: bias + output slices.
            xc_load(0, 0, 1)
            for gob in range(G):
                t = w_alloc(0, gob)
                nc.gpsimd.dma_start(t[:, 0:4, :], wc_d[0, gob, :, 0:4, :])
            xc_load(0, 1, 1)
            xc_load(0, 2, 1)
            xc_load(0, 3, 1)
            for gob in range(G):
                nc.gpsimd.dma_start(
                    w_tiles[(0, gob)][:, 4:8, :], wc_d[0, gob, :, 4:8, :])
            xc_load(0, 4, 2)
            xc_load(0, 6, 2)
            for gob in range(G):
                nc.gpsimd.dma_start(
                    w_tiles[(0, gob)][:, 8:16, :], wc_d[0, gob, :, 8:16, :])
            xc_load(0, 8, 4)
            xc_load(0, 12, 4)
            nc.sync.dma_start(b_sb[:], b_d[:])
            # second DMA queue (sync, idle at startup) carries half the
            # x-combos so the 14.7 MB x-side lands in half the time
            xc_load(1, 0, 4, eng=nc.sync)
            xc_load(1, 4, 4, eng=nc.sync)
            xc_load(2, 0, 4)
            xc_load(2, 4, 4)
            for j in range(1, NPROD):
                for gob in range(G):
                    w_load(j, gob)
            xc_load(1, 8, 8, eng=nc.sync)
            xc_load(2, 8, 8)
            xc_load(3, 0, 8, eng=nc.sync)
            xc_load(3, 8, 8, eng=nc.sync)
            xc_load(4, 0, 16)
            xc_load(5, 0, 16, eng=nc.sync)
            xc_load(6, 0, 16)

            # startup W for the first steady blocks so obp=G starts clean
            for j in range(NPROD):
                w_load(j, G)

            # ---- PE warmup dummies ----
            ps_warm = pp.tile([P, TH], dt.float32, tag="ps", name="ps_w")
            for _ in range(14):
                nc.tensor.matmul(
                    ps_warm[:, 0:256], dum[:, 0:128], dum[:, 0:256],
                    start=True, stop=True,
                )

            stage = {}

            def evict(j, obp, ps):
                m = mp.tile([P, TH], dt.float16, tag="m", name=f"m_{j}_{obp}")
                nc.scalar.copy(m[:], ps[:])
                stage[(j, obp)] = m
                return m

            def product(j, obp, ps=None):
                if ps is None:
                    ps = pp.tile([P, TH], dt.float32, tag="ps",
                                 name=f"ps_{j}_{obp}")
                wt = w_tiles.pop((j, obp))
                for c in range(KH):
                    nc.tensor.matmul(
                        ps[:], wt[:, c, :], xc_sb[j][:, c, :],
                        start=(c == 0), stop=(c == KH - 1),
                    )
                evict(j, obp, ps)

            # Incremental recombine: emit each scalar_tensor_tensor as soon
            # as its staged inputs exist (called with the just-finished j),
            # all on the vector ALU (gpsimd/Pool lacks TensorScalarPtr on NC-v3).  After the last
            # product of an o-block only evict -> t2 -> C11 -> DMA remains.
            #   C11 = M1+M4-M5+M7+bt   C12 = M3+M5+bt
            #   C21 = M2+M4+bb         C22 = M1-M2+M3+M6+bb
            rec = {}

            def recombine_step(obp, j):
                bt = b_sb[:, obp:obp + 1]
                bb = b_sb[:, OBH + obp:OBH + obp + 1]
                m = lambda k: stage[(k, obp)]
                rt = slice(obp * P, (obp + 1) * P)
                rb = slice((OBH + obp) * P, (OBH + obp + 1) * P)
                r = rec.setdefault(obp, {})

                def tl(pool, tag, nm):
                    return pool.tile([P, TH], dt.float16, tag=tag,
                                     name=f"{nm}_{obp}")

                if j == 1:
                    r["t3"] = tl(tp, "t", "t3")
                    nc.vector.scalar_tensor_tensor(
                        r["t3"][:], m(0)[:], bb, m(1)[:], alu.add, alu.subtract)
                elif j == 3:
                    r["t1"] = tl(tp, "t", "t1")
                    nc.vector.scalar_tensor_tensor(
                        r["t1"][:], m(0)[:], bt, m(3)[:], alu.add, alu.add)
                    o21 = tl(op, "o", "o21")
                    nc.vector.scalar_tensor_tensor(
                        o21[:], m(1)[:], bb, m(3)[:], alu.add, alu.add)
                    nc.sync.dma_start(y_d[rb, 0:TH], o21[:])
                elif j == 4:
                    o12 = tl(op, "o", "o12")
                    nc.vector.scalar_tensor_tensor(
                        o12[:], m(2)[:], bt, m(4)[:], alu.add, alu.add)
                    nc.sync.dma_start(y_d[rt, TH:T_CORE], o12[:])
                elif j == 5:
                    t4 = tl(tp, "t", "t4")
                    nc.vector.scalar_tensor_tensor(
                        t4[:], m(2)[:], 0.0, m(5)[:], alu.add, alu.add)
                    o22 = tl(op, "o", "o22")
                    nc.vector.scalar_tensor_tensor(
                        o22[:], r["t3"][:], 0.0, t4[:], alu.add, alu.add)
                    nc.sync.dma_start(y_d[rb, TH:T_CORE], o22[:])
                elif j == 6:
                    t2 = tl(tp, "t", "t2")
                    nc.vector.scalar_tensor_tensor(
                        t2[:], m(6)[:], 0.0, m(4)[:], alu.add, alu.subtract)
                    o11 = tl(op, "o", "o11")
                    nc.vector.scalar_tensor_tensor(
                        o11[:], r["t1"][:], 0.0, t2[:], alu.add, alu.add)
                    nc.sync.dma_start(y_d[rt, 0:TH], o11[:])
                    for k in range(NPROD):
                        del stage[(k, obp)]
                    del rec[obp]

            # ---- startup group: j-major, c-outer across obp 0..G-1 ----
            for j in range(NPROD):
                ps_j = []
                for gob in range(G):
                    if j == 0 and gob == 0:
                        ps_j.append(ps_warm)
                    else:
                        ps_j.append(pp.tile(
                            [P, TH], dt.float32, tag="ps",
                            name=f"ps_{j}_{gob}"))
                for c in range(KH):
                    for gob in range(G):
                        nc.tensor.matmul(
                            ps_j[gob][:],
                            w_tiles[(j, gob)][:, c, :], xc_sb[j][:, c, :],
                            start=(c == 0), stop=(c == KH - 1),
                        )
                for gob in range(G):
                    evict(j, gob, ps_j[gob])
                for gob in range(G):
                    recombine_step(gob, j)
            for j, gob in list(w_tiles):
                if gob < G:
                    del w_tiles[(j, gob)]

            # ---- steady state: obp-major ----
            for obp in range(G, OBH):
                for j in range(NPROD):
                    if obp + 1 < OBH:
                        w_load(j, obp + 1)
                    product(j, obp)
                    recombine_step(obp, j)

    nc.compile()
    return nc


def _get_nc():
    global _compiled
    if _compiled is None:
        _compiled = _build_nc()
    return _compiled


def _prep_inputs(x, W, b):
    x = np.asarray(x, dtype=np.float32)
    W = np.asarray(W, dtype=np.float32)
    b = np.asarray(b, dtype=np.float32)

    Wr = _matmul_hadU_np(W.astype(np.float64))  # [o, k] float64
    A11 = Wr[:HALF, :HALF]
    A12 = Wr[:HALF, HALF:]
    A21 = Wr[HALF:, :HALF]
    A22 = Wr[HALF:, HALF:]
    WCs = (A11 + A22, A21 + A22, A11, A22, A11 + A12, A21 - A11, A12 - A22)
    # pack[j][obp, p, c, jo] = WC_j[obp*128 + jo, c*128 + p]
    wc = np.stack([
        w.reshape(OBH, P, KH, P).transpose(0, 3, 2, 1) for w in WCs
    ]).astype(np.float16)
    wc = np.ascontiguousarray(wc)

    b_pack = np.ascontiguousarray(b.reshape(2 * OBH, P).T)  # [128, 32]

    xt = x.reshape(N_CORES, T_CORE, D).transpose(0, 2, 1)  # [core, k, t] f32
    B11 = xt[:, :HALF, :TH]
    B12 = xt[:, :HALF, TH:]
    B21 = xt[:, HALF:, :TH]
    B22 = xt[:, HALF:, TH:]
    XCs = (B11 + B22, B11, B12 - B22, B21 - B11, B22, B11 + B12, B21 + B22)
    # pack[core, j, p, c, t] = XC_j[core, c*128 + p, t]
    xc = np.stack([
        c.reshape(N_CORES, KH, P, TH).transpose(0, 2, 1, 3) for c in XCs
    ], axis=1).astype(np.float16)
    xc = np.ascontiguousarray(xc)

    in_maps = [
        {"xc": xc[i], "wc": wc, "bias": b_pack} for i in range(N_CORES)
    ]
    return in_maps


def _assemble(results):
    # yt per core: [4096 o, 1024 t] fp16 -> y[t, o] fp32
    parts = [r["yt"].T.astype(np.float32) for r in results]
    y = np.concatenate(parts, axis=0)  # [8192, 4096]
    return y.reshape(4, 2048, D)


def _run(x, W, b, **spmd_kwargs):
    from concourse.bass_utils import run_bass_kernel_spmd

    nc = _get_nc()
    in_maps = _prep_inputs(x, W, b)
    res = run_bass_kernel_spmd(nc, in_maps, list(range(N_CORES)), **spmd_kwargs)
    return _assemble(res.results), res


def kernel(x, W, b):
    out, _ = _run(x, W, b)
    return out


# revision 16
# speedup vs baseline: 1.0667x; 1.0139x over previous
"""Trainium2 Bass kernel for nn_InputRotationWrapper: y = WHT(x) @ W^T + b.

Algebraic fold: WHT (normalized Walsh-Hadamard along feature dim, H symmetric)
commutes into the weight: y = (x H) W^T = x (W H)^T.  The device runs a pure
GEMM  y = x @ Wr^T + b  with Wr = WHT(W) computed once on the host.

On top of the fold, one level of STRASSEN over 2x2x2 blocking of
(o, k, t) cuts the PE matmul count by 1/8 — the kernel is PE-streaming-bound
at fp16 (1 moving column/cycle), so this is a direct 12.5% win that neither
fp8 (accuracy: e4m3 x,W measures 3.8e-2 rel err vs the 2e-2 gate) nor uint8
(TRN2 silicon zeroes integer matmul products; probed via NEFF dtype patch)
can reach.

  C = Wr @ x^T = [[C11 C12],[C21 C22]],  A = Wr halves, B = x^T halves
  M1=(A11+A22)(B11+B22) M2=(A21+A22)B11 M3=A11(B12-B22) M4=A22(B21-B11)
  M5=(A11+A12)B22 M6=(A21-A11)(B11+B12) M7=(A12-A22)(B21+B22)
  C11=M1+M4-M5+M7  C12=M3+M5  C21=M2+M4  C22=M1-M2+M3+M6

All 7 A-combos (W-side) and 7 B-combos (x-side) are precomputed on the host
in f64/f32 and shipped as fp16: the device only runs products and cheap
recombines.  Per core (1024 tokens, data-parallel over 8 cores):

  - 7 x-combos resident in SBUF: [128p, 16c, 512t] fp16 each (14.7 MB)
  - W-combos streamed per (product j, o-block obp): [128p, 16c, 128o] fp16
  - 16 obp iterations x 7 products x 16-chunk PSUM accumulation
    = 1792 matmuls of 512 cols (vs 2048 classical) ~ 387 us PE wall
  - ScalarE evicts each product PSUM->SBUF fp16; VectorE recombines with
    scalar_tensor_tensor (bias fused via the per-partition scalar operand);
    outputs DMA per [128, 512] slice.  All hidden under PE time.

Startup mirrors the fp16 baseline: PE-clock warmup dummies, then a j-major
group over the first G o-blocks processed c-outer so every arriving x-combo
chunk immediately unlocks G matmuls while the DMA subsystem ramps.
"""
import sys

for _p in ("/opt/trn_rl_repo", "/root/.axon_site/_ro/trn_rl_repo"):
    if _p not in sys.path:
        sys.path.insert(0, _p)

import numpy as np

D = 4096          # feature dim (= rotation size)
TOKENS = 8192     # 4 * 2048
N_CORES = 8
T_CORE = TOKENS // N_CORES   # 1024 tokens per core
P = 128           # partitions
HALF = D // 2     # 2048: o/k half size
KH = HALF // P    # 16 contraction chunks per half
OBH = HALF // P   # 16 output blocks per half
TH = T_CORE // 2  # 512 tokens per t-half (= one matmul moving dim)
NPROD = 7
ORDER = (0, 1, 2, 3, 4, 6, 5)  # product emission order (M6 last: 1-stt tail)

_compiled = None


def _matmul_hadU_np(x: np.ndarray) -> np.ndarray:
    """Normalized WHT along the last axis — exact port of the reference
    recursive-butterfly (K == 1 branch), in float64."""
    n = x.shape[-1]
    shape = x.shape
    v = x.reshape(-1, n, 1)
    while v.shape[1] > 1:
        b_, m, c = v.shape
        v = v.reshape(b_, m // 2, 2, c)
        a, b = v[:, :, 0, :], v[:, :, 1, :]
        v = np.concatenate([a + b, a - b], axis=-1)
    return v.reshape(shape) / np.sqrt(n)


def _build_nc():
    import concourse.tile as tile
    from concourse import bacc, mybir

    dt = mybir.dt
    alu = mybir.AluOpType
    nc = bacc.Bacc(None, target_bir_lowering=False)

    xc_d = nc.dram_tensor("xc", [NPROD, P, KH, TH], dt.float16,
                          kind="ExternalInput")
    wc_d = nc.dram_tensor("wc", [NPROD, OBH, P, KH, P], dt.float16,
                          kind="ExternalInput")
    b_d = nc.dram_tensor("bias", [P, 2 * OBH], dt.float32,
                         kind="ExternalInput")
    y_d = nc.dram_tensor("yt", [D, T_CORE], dt.float16, kind="ExternalOutput")

    G = 4     # startup group: o-blocks processed c-outer per product so each
              # arriving x-combo chunk unlocks G matmuls during the DMA ramp
    WRING = 8   # W tile ring (4 KB/partition each)
    MRING = 24  # staged-product ring (1 KB/partition each); incremental
                # recombine frees most of the startup group by j=3

    with tile.TileContext(nc) as tc:
        with (
            tc.tile_pool(name="xcp", bufs=1) as xcp,
            tc.tile_pool(name="wp", bufs=WRING) as wp,
            tc.tile_pool(name="mp", bufs=MRING) as mp,
            tc.tile_pool(name="tp", bufs=12) as tp,
            tc.tile_pool(name="op", bufs=4) as op,
            tc.tile_pool(name="bp", bufs=1) as bp,
            tc.tile_pool(name="pp", bufs=8, space="PSUM") as pp,
        ):
            b_sb = bp.tile([P, 2 * OBH], dt.float32)

            xc_sb = [
                xcp.tile([P, KH, TH], dt.float16, name=f"xc_{j}")
                for j in range(NPROD)
            ]

            # ---- PE clock warmup (HAM ramps over ~3.4us of activity) ----
            dum = bp.tile([P, 256], dt.float16, tag="dum", name="dum")
            nc.vector.memset(dum[:], 0.0)

            w_tiles = {}

            def w_alloc(j, obp):
                t = wp.tile([P, KH, P], dt.float16, tag="w",
                            name=f"w_{j}_{obp}")
                w_tiles[(j, obp)] = t
                return t

            def w_load(j, obp, eng=None):
                t = w_alloc(j, obp)
                (eng or nc.gpsimd).dma_start(t[:], wc_d[j, obp, :, :, :])
                return t

            def xc_load(j, c0, n, eng=None):
                (eng or nc.scalar).dma_start(
                    xc_sb[j][:, c0:c0 + n, :], xc_d[j, :, c0:c0 + n, :])

            # ---- DMA triggers in arrival-need order ----
            # A single HWDGE queue sustains only ~146 GB/s, so the 58.7 MB W
            # stream is split across two queues (even products on gpsimd,
            # odd on scalar ~ 82/62 GB/s each) and each x-combo is split in
            # c-halves between the scalar and sync queues.
            def w_eng(j):
                return nc.scalar if j % 2 else nc.gpsimd

            xc_load(0, 0, 1)
            for gob in range(G):
                t = w_alloc(0, gob)
                nc.gpsimd.dma_start(t[:, 0:4, :], wc_d[0, gob, :, 0:4, :])
            xc_load(0, 1, 1)
            xc_load(0, 2, 1)
            xc_load(0, 3, 1)
            for gob in range(G):
                nc.gpsimd.dma_start(
                    w_tiles[(0, gob)][:, 4:8, :], wc_d[0, gob, :, 4:8, :])
            nc.sync.dma_start(b_sb[:], b_d[:])
            xc_load(0, 8, 8, eng=nc.sync)
            xc_load(0, 4, 2)
            xc_load(0, 6, 2)
            for gob in range(G):
                nc.gpsimd.dma_start(
                    w_tiles[(0, gob)][:, 8:16, :], wc_d[0, gob, :, 8:16, :])
            # startup group W + first x-combo halves, in ORDER position need
            for gob in range(G):
                w_load(1, gob, eng=nc.scalar)
            xc_load(1, 0, 4)
            xc_load(1, 8, 8, eng=nc.sync)
            xc_load(1, 4, 4)
            for gob in range(G):
                w_load(2, gob, eng=nc.gpsimd)
            xc_load(2, 0, 8)
            xc_load(2, 8, 8, eng=nc.sync)
            for gob in range(G):
                w_load(3, gob, eng=nc.scalar)
            xc_load(3, 0, 8)
            xc_load(3, 8, 8, eng=nc.sync)
            for gob in range(G):
                w_load(4, gob, eng=nc.gpsimd)
            xc_load(4, 0, 16, eng=nc.sync)
            for gob in range(G):
                w_load(6, gob, eng=nc.gpsimd)
            xc_load(6, 0, 16, eng=nc.sync)
            for gob in range(G):
                w_load(5, gob, eng=nc.scalar)
            xc_load(5, 0, 16, eng=nc.sync)

            # startup W for the first steady block so obp=G starts clean
            for j in ORDER:
                w_load(j, G, eng=w_eng(j))

            # ---- PE warmup dummies ----
            ps_warm = pp.tile([P, TH], dt.float32, tag="ps", name="ps_w")
            for _ in range(14):
                nc.tensor.matmul(
                    ps_warm[:, 0:256], dum[:, 0:128], dum[:, 0:256],
                    start=True, stop=True,
                )

            stage = {}

            def evict(j, obp, ps):
                m = mp.tile([P, TH], dt.float16, tag="m", name=f"m_{j}_{obp}")
                nc.scalar.copy(m[:], ps[:])
                stage[(j, obp)] = m
                return m

            def product(j, obp, ps=None):
                if ps is None:
                    ps = pp.tile([P, TH], dt.float32, tag="ps",
                                 name=f"ps_{j}_{obp}")
                wt = w_tiles.pop((j, obp))
                for c in range(KH):
                    nc.tensor.matmul(
                        ps[:], wt[:, c, :], xc_sb[j][:, c, :],
                        start=(c == 0), stop=(c == KH - 1),
                    )
                evict(j, obp, ps)

            # Incremental recombine: emit each scalar_tensor_tensor as soon
            # as its staged inputs exist (called with the just-finished j),
            # all on the vector ALU (gpsimd/Pool lacks TensorScalarPtr on
            # NC-v3).  Products run in ORDER = [0,1,2,3,4,6,5] and C22 is
            # built as (M1-M2+bb) + M3 earlier, so after the LAST product of
            # every o-block only evict -> one stt -> DMA remains.
            #   C11 = M1+M4-M5+M7+bt   C12 = M3+M5+bt
            #   C21 = M2+M4+bb         C22 = ((M1-M2+bb) + M3) + M6
            rec = {}

            def recombine_step(obp, j):
                bt = b_sb[:, obp:obp + 1]
                bb = b_sb[:, OBH + obp:OBH + obp + 1]
                m = lambda k: stage[(k, obp)]
                rt = slice(obp * P, (obp + 1) * P)
                rb = slice((OBH + obp) * P, (OBH + obp + 1) * P)
                r = rec.setdefault(obp, {})

                def tl(pool, tag, nm):
                    return pool.tile([P, TH], dt.float16, tag=tag,
                                     name=f"{nm}_{obp}")

                if j == 1:
                    r["t3"] = tl(tp, "t", "t3")
                    nc.vector.scalar_tensor_tensor(
                        r["t3"][:], m(0)[:], bb, m(1)[:], alu.add, alu.subtract)
                elif j == 2:
                    r["t5"] = tl(tp, "t", "t5")
                    nc.vector.scalar_tensor_tensor(
                        r["t5"][:], r["t3"][:], 0.0, m(2)[:], alu.add, alu.add)
                elif j == 3:
                    r["t1"] = tl(tp, "t", "t1")
                    nc.vector.scalar_tensor_tensor(
                        r["t1"][:], m(0)[:], bt, m(3)[:], alu.add, alu.add)
                    o21 = tl(op, "o", "o21")
                    nc.vector.scalar_tensor_tensor(
                        o21[:], m(1)[:], bb, m(3)[:], alu.add, alu.add)
                    nc.sync.dma_start(y_d[rb, 0:TH], o21[:])
                elif j == 4:
                    o12 = tl(op, "o", "o12")
                    nc.vector.scalar_tensor_tensor(
                        o12[:], m(2)[:], bt, m(4)[:], alu.add, alu.add)
                    nc.sync.dma_start(y_d[rt, TH:T_CORE], o12[:])
                elif j == 6:
                    t2 = tl(tp, "t", "t2")
                    nc.vector.scalar_tensor_tensor(
                        t2[:], m(6)[:], 0.0, m(4)[:], alu.add, alu.subtract)
                    o11 = tl(op, "o", "o11")
                    nc.vector.scalar_tensor_tensor(
                        o11[:], r["t1"][:], 0.0, t2[:], alu.add, alu.add)
                    nc.sync.dma_start(y_d[rt, 0:TH], o11[:])
                elif j == 5:
                    o22 = tl(op, "o", "o22")
                    nc.vector.scalar_tensor_tensor(
                        o22[:], r["t5"][:], 0.0, m(5)[:], alu.add, alu.add)
                    nc.sync.dma_start(y_d[rb, TH:T_CORE], o22[:])
                    for k in range(NPROD):
                        del stage[(k, obp)]
                    del rec[obp]

            # ---- startup group: j-major, c-outer across obp 0..G-1 ----
            for j in ORDER:
                ps_j = []
                for gob in range(G):
                    if j == 0 and gob == 0:
                        ps_j.append(ps_warm)
                    else:
                        ps_j.append(pp.tile(
                            [P, TH], dt.float32, tag="ps",
                            name=f"ps_{j}_{gob}"))
                for c in range(KH):
                    for gob in range(G):
                        nc.tensor.matmul(
                            ps_j[gob][:],
                            w_tiles[(j, gob)][:, c, :], xc_sb[j][:, c, :],
                            start=(c == 0), stop=(c == KH - 1),
                        )
                for gob in range(G):
                    evict(j, gob, ps_j[gob])
                for gob in range(G):
                    recombine_step(gob, j)
            for j, gob in list(w_tiles):
                if gob < G:
                    del w_tiles[(j, gob)]

            # ---- steady state: obp-major ----
            for obp in range(G, OBH):
                for j in ORDER:
                    if obp + 1 < OBH:
                        w_load(j, obp + 1, eng=w_eng(j))
                    product(j, obp)
                    recombine_step(obp, j)

    nc.compile()
    return nc


def _get_nc():
    global _compiled
    if _compiled is None:
        _compiled = _build_nc()
    return _compiled


def _prep_inputs(x, W, b):
    x = np.asarray(x, dtype=np.float32)
    W = np.asarray(W, dtype=np.float32)
    b = np.asarray(b, dtype=np.float32)

    Wr = _matmul_hadU_np(W.astype(np.float64))  # [o, k] float64
    A11 = Wr[:HALF, :HALF]
    A12 = Wr[:HALF, HALF:]
    A21 = Wr[HALF:, :HALF]
    A22 = Wr[HALF:, HALF:]
    WCs = (A11 + A22, A21 + A22, A11, A22, A11 + A12, A21 - A11, A12 - A22)
    # pack[j][obp, p, c, jo] = WC_j[obp*128 + jo, c*128 + p]
    wc = np.stack([
        w.reshape(OBH, P, KH, P).transpose(0, 3, 2, 1) for w in WCs
    ]).astype(np.float16)
    wc = np.ascontiguousarray(wc)

    b_pack = np.ascontiguousarray(b.reshape(2 * OBH, P).T)  # [128, 32]

    xt = x.reshape(N_CORES, T_CORE, D).transpose(0, 2, 1)  # [core, k, t] f32
    B11 = xt[:, :HALF, :TH]
    B12 = xt[:, :HALF, TH:]
    B21 = xt[:, HALF:, :TH]
    B22 = xt[:, HALF:, TH:]
    XCs = (B11 + B22, B11, B12 - B22, B21 - B11, B22, B11 + B12, B21 + B22)
    # pack[core, j, p, c, t] = XC_j[core, c*128 + p, t]
    xc = np.stack([
        c.reshape(N_CORES, KH, P, TH).transpose(0, 2, 1, 3) for c in XCs
    ], axis=1).astype(np.float16)
    xc = np.ascontiguousarray(xc)

    in_maps = [
        {"xc": xc[i], "wc": wc, "bias": b_pack} for i in range(N_CORES)
    ]
    return in_maps


def _assemble(results):
    # yt per core: [4096 o, 1024 t] fp16 -> y[t, o] fp32
    parts = [r["yt"].T.astype(np.float32) for r in results]
    y = np.concatenate(parts, axis=0)  # [8192, 4096]
    return y.reshape(4, 2048, D)


def _run(x, W, b, **spmd_kwargs):
    from concourse.bass_utils import run_bass_kernel_spmd

    nc = _get_nc()
    in_maps = _prep_inputs(x, W, b)
    res = run_bass_kernel_spmd(nc, in_maps, list(range(N_CORES)), **spmd_kwargs)
    return _assemble(res.results), res


def kernel(x, W, b):
    out, _ = _run(x, W, b)
    return out
